# revision 24
# baseline (speedup 1.0000x reference)
"""Multi-head attention (B=4, S=2048, E=1024, H=16, D=64) on 8 trn2 cores.

Sharding: 2D (batch x head-group). Core c handles batch b = c//2 and head
group g = c%2 (8 heads = 512 feature dims). Each core computes a full
[S, E] partial of the output projection for its batch; the host sums the
two group partials per batch and adds the bias.

Per-core kernel (v2 — head-pair concurrent QK + fine-grained interleave):
  qT/kT: W @ X^T computed in [oc chunk, S-block] pieces -> [512, 2048] fp16
  v: X_v @ Wv^T per 128-token chunk (+ ones column, bf16)
  attention in head PAIRS (A at partitions 0-63, B at 64-127):
    per (pair, qt=512 block, kk chunk):
      sA = kT_A_chunk.T @ qT_A[qt]   (64-contract row-tile (0,0))
      sB = kT_B_chunk.T @ qT_B[qt]   (row-tile (64,0), runs CONCURRENT)
      -> one PSUM tile S2 [128, 1024] = [sA | sB]
      P2 = exp(S2)  (one ACT instr, FD=1024) -> SBUF bf16
      U_A += v_aug_A.T @ P2[:, 0:512]   (row 64 = softmax denom)
      U_B += v_aug_B.T @ P2[:, 512:1024]
    aT = U[0:64] / U[64]  per head
  y = aT.T @ Wo  -> [2048, 1024] fp16 partial, summed on host.

Projection work is queued as small steps (~4 matmuls) and drained 1-2
steps per kk iteration so the PE fills ACT-bound slack without starving
the exp stream.
"""

from collections import deque
from contextlib import ExitStack

import numpy as np

S = 2048
E = 1024
F = 512          # local feature dims (8 heads x 64)
HL = 8           # heads per core
D = 64
B = 4
H = 16
NCORES = 8
QT = 512         # q block
NQT = S // QT    # 4
NKK = S // 128   # 16

_CACHE = {}


def build_nc(reps: int = 1):
    import concourse.tile as tile
    from concourse import bacc, mybir

    F16 = mybir.dt.float16
    BF16 = mybir.dt.bfloat16
    F32 = mybir.dt.float32
    EXP = mybir.ActivationFunctionType.Exp

    nc = bacc.Bacc(
        "TRN2",
        target_bir_lowering=False,
        debug=False,
        enable_asserts=False,
        num_devices=NCORES,
    )

    xq_d = nc.dram_tensor("xq", [E, S], F16, kind="ExternalInput").ap()
    xk_d = nc.dram_tensor("xk", [E, S], F16, kind="ExternalInput").ap()
    xv_d = nc.dram_tensor("xv", [E, S], F16, kind="ExternalInput").ap()
    wq_d = nc.dram_tensor("wq", [E, F], F16, kind="ExternalInput").ap()
    wk_d = nc.dram_tensor("wk", [E, F], F16, kind="ExternalInput").ap()
    wv_d = nc.dram_tensor("wv", [E, F], F16, kind="ExternalInput").ap()
    wo_d = nc.dram_tensor("wo", [F, E], F16, kind="ExternalInput").ap()
    y_d = nc.dram_tensor("y", [S, E], F16, kind="ExternalOutput").ap()

    with tile.TileContext(nc) as tc, ExitStack() as ctx:
        persist = ctx.enter_context(tc.tile_pool(name="persist", bufs=1))
        xqk = ctx.enter_context(tc.tile_pool(name="xqk", bufs=1))
        xvp = ctx.enter_context(tc.tile_pool(name="xvp", bufs=6))
        p2p = ctx.enter_context(tc.tile_pool(name="p2p", bufs=4))
        ypool = ctx.enter_context(tc.tile_pool(name="ypool", bufs=2))
        smpool = ctx.enter_context(tc.tile_pool(name="smpool", bufs=2))
        ps_s = ctx.enter_context(tc.tile_pool(name="ps_s", bufs=2, space="PSUM"))
        ps_u = ctx.enter_context(tc.tile_pool(name="ps_u", bufs=2, space="PSUM"))
        ps_pp = ctx.enter_context(tc.tile_pool(name="ps_pp", bufs=2, space="PSUM"))

        def body(iv):
            # ---------------- persistent weights ----------------
            def load_w(dram, pfx, width):
                tiles = []
                for i in range(dram.shape[0] // 128):
                    t = persist.tile([128, width], F16, tag=f"{pfx}{i}",
                                     name=f"{pfx}_sb{i}")
                    nc.sync.dma_start(t[:], dram[i * 128:(i + 1) * 128, :])
                    tiles.append(t)
                return tiles

            # DMA granularity tracks first consumption: x for q/k arrives
            # in [eci, S-block] pieces (first scores after ~4MB, not 10MB);
            # xv in per-token-chunk strided pieces; wo last.
            wq_sb = load_w(wq_d, "wq", F)
            xqk_tiles = {}

            def load_xsb(pfx, sb):
                """Load the 8 [128, QT] eci tiles of one S-block of xq/xk."""
                if (pfx, sb) in xqk_tiles:
                    return xqk_tiles[(pfx, sb)]
                x_d = xq_d if pfx == "q" else xk_d
                tiles = []
                for eci in range(8):
                    t = xqk.tile([128, QT], F16, tag=f"x{pfx}{sb}_{eci}",
                                 name=f"x{pfx}{sb}_{eci}")
                    nc.sync.dma_start(
                        t[:], x_d[eci * 128:(eci + 1) * 128,
                                  sb * QT:(sb + 1) * QT])
                    tiles.append(t)
                xqk_tiles[(pfx, sb)] = tiles
                return tiles

            load_xsb("q", 0)
            wk_sb = load_w(wk_d, "wk", F)
            load_xsb("k", 0)
            wv_sb = load_w(wv_d, "wv", F)

            # v with ones column: v_sb[p, tc, h, d] = v[tc*128+p, h*64+d],
            # d=64 column stays 1.0 (softmax denominator trick)
            v_sb = persist.tile([128, NKK, HL, D + 1], BF16, tag="v_sb",
                                name="v_sb")
            nc.vector.memset(v_sb[:], 1.0)

            qT_sb = [persist.tile([128, S], F16, tag=f"qT{i}", name=f"qT_sb{i}")
                     for i in range(4)]
            kT_sb = [persist.tile([128, S], F16, tag=f"kT{i}", name=f"kT_sb{i}")
                     for i in range(4)]
            aT_sb = [persist.tile([128, S], F16, tag=f"aT{i}", name=f"aT_sb{i}")
                     for i in range(4)]
            wo_sb = load_w(wo_d, "wo", E)

            # ---------------- projection work queue ----------------
            # Each step is a closure issuing ~4 matmuls (~0.9us of PE).
            work = deque()

            def emit_qk(w_sb, out_tiles, pfx, sb, oc):
                def load():
                    load_xsb(pfx, sb)

                def mms(first):
                    pp = mms.pp
                    x_sb = xqk_tiles[(pfx, sb)]
                    for eci in (range(0, 4) if first else range(4, 8)):
                        nc.tensor.matmul(
                            pp[:],
                            lhsT=w_sb[eci][:, oc * 128:(oc + 1) * 128],
                            rhs=x_sb[eci][:],
                            start=(eci == 0),
                            stop=(eci == 7),
                        )
                    if not first:
                        nc.vector.tensor_copy(
                            out_tiles[oc][:, sb * QT:(sb + 1) * QT], pp[:])

                def step1():
                    mms.pp = ps_pp.tile([128, QT], F32, tag="pp",
                                        name=f"pp{pfx}{oc}_{sb}")
                    mms(True)

                return load, step1, (lambda: mms(False))

            # v projection per 128-token chunk tc: xv column chunk arrives
            # as one strided DMA [128, (eci, 128)].
            def emit_v(tc_i):
                xt = [None]

                def load():
                    t = xvp.tile([128, E], F16, tag="xv", name=f"xv{tc_i}")
                    vsrc = xv_d[:, tc_i * 128:(tc_i + 1) * 128].rearrange(
                        "(e p) t -> p e t", e=8)
                    vdst = t[:].rearrange("p (e t) -> p e t", e=8)
                    nc.sync.dma_start(vdst, vsrc)
                    xt[0] = t

                def mms(first):
                    pp = mms.pp
                    for eci in (range(0, 4) if first else range(4, 8)):
                        nc.tensor.matmul(
                            pp[:],
                            lhsT=xt[0][:, eci * 128:(eci + 1) * 128],
                            rhs=wv_sb[eci][:],
                            start=(eci == 0),
                            stop=(eci == 7),
                        )
                    if not first:
                        # strided copy into [tc, h, 0:64] slots
                        nc.vector.tensor_copy(v_sb[:, tc_i, :, 0:D], pp[:])

                def step1():
                    mms.pp = ps_pp.tile([128, QT], F32, tag="pp",
                                        name=f"ppv{tc_i}")
                    mms(True)

                return load, step1, (lambda: mms(False))

            # output projection per 128-row chunk tci (needs all aT);
            # each 512-col half holds one ps_pp slot only within its step
            def emit_y(tci):
                ysb_ref = [None]

                def step(nb):
                    if nb == 0:
                        ysb_ref[0] = ypool.tile([128, E], F16, tag="y",
                                                name=f"y_sb{tci}")
                    pp = ps_pp.tile([128, QT], F32, tag="pp",
                                    name=f"ppy{tci}_{nb}")
                    for fc in range(4):
                        nc.tensor.matmul(
                            pp[:],
                            lhsT=aT_sb[fc][:, tci * 128:(tci + 1) * 128],
                            rhs=wo_sb[fc][:, nb * QT:(nb + 1) * QT],
                            start=(fc == 0),
                            stop=(fc == 3),
                        )
                    ysb = ysb_ref[0]
                    nc.vector.tensor_copy(
                        ysb[:, nb * QT:(nb + 1) * QT], pp[:])
                    if nb == 1:
                        nc.sync.dma_start(
                            y_d[tci * 128:(tci + 1) * 128, :], ysb[:])

                return (lambda: step(0)), (lambda: step(1))

            # Build the queue in dependency-need order.
            # 1) qk oc0/sb0 upfront (pair0/qt0 needs it) -- issued eagerly.
            # 2) v chunks (pair0 AV consumes tc=kk at step kk).
            # 3) qk oc0 sb1-3 (pair0 qt1-3), oc1.. before later pairs.
            qk_steps = {}
            for oc in range(4):
                for sb in range(NQT):
                    for pfx, w_sb, outs in (("q", wq_sb, qT_sb),
                                            ("k", wk_sb, kT_sb)):
                        ld, s1, s2 = emit_qk(w_sb, outs, pfx, sb, oc)
                        qk_steps[(pfx, oc, sb)] = (ld, s1, s2)
            v_steps = {tc_i: emit_v(tc_i) for tc_i in range(NKK)}

            # issue order of proj steps under the qt-outer schedule
            # (block index = qt*4 + ch, 16 pumps each):
            #   v[tc] due pump tc (block 0's AV); kT oc needs ALL S
            #   columns by block (0, oc); qT oc needs col-block qt by
            #   block (qt, oc).
            order = []
            order.extend(v_steps[t] for t in range(3))
            for sb in range(1, NQT):
                order.append(qk_steps[("k", 0, sb)])
                order.extend(v_steps[t] for t in range(4 * sb - 1,
                                                       4 * sb + 3))
            order.append(v_steps[15])
            for oc in range(1, 4):
                order.append(qk_steps[("q", oc, 0)])
                for sb in range(NQT):
                    order.append(qk_steps[("k", oc, sb)])
            for sb in range(1, NQT):
                for oc in range(4):
                    order.append(qk_steps[("q", oc, sb)])

            # prologue: first q/k block inline so attention can start
            for key in (("q", 0, 0), ("k", 0, 0)):
                ld, s1, s2 = qk_steps[key]
                ld()
                s1()
                s2()
            pending = deque([[ld, s1, s2, False] for ld, s1, s2 in order])

            def _prefetch(depth=5):
                for i, e in enumerate(pending):
                    if i >= depth:
                        break
                    if not e[3]:
                        e[0]()
                        e[3] = True

            _prefetch()

            def pump(n):
                for _ in range(n):
                    if work:
                        work.popleft()()
                    elif pending:
                        e = pending.popleft()
                        if not e[3]:
                            e[0]()
                            e[3] = True
                        e[1]()
                        work.append(e[2])
                        _prefetch()
                    else:
                        return

            # ---------------- attention ----------------
            def attn_pair(ch, qt, pump_n):
                hA, hB = 2 * ch, 2 * ch + 1
                U = [ps_u.tile([D + 1, QT], F32, tag="u", name=f"U{ch}_{qt}_{i}")
                     for i in range(2)]
                qcol = qt * QT
                def av(kk, p2):
                    nc.tensor.matmul(
                        U[0][:],
                        lhsT=v_sb[:, kk, hA, :],
                        rhs=p2[:, 0:QT],
                        start=(kk == 0), stop=(kk == NKK - 1),
                    )
                    nc.tensor.matmul(
                        U[1][:],
                        lhsT=v_sb[:, kk, hB, :],
                        rhs=p2[:, QT:2 * QT],
                        start=(kk == 0), stop=(kk == NKK - 1),
                    )

                prev = None
                for kk in range(NKK):
                    s2 = ps_s.tile([128, 2 * QT], F32, tag="s",
                                   name=f"s{ch}_{qt}_{kk}")
                    kcol = kk * 128
                    nc.tensor.matmul(
                        s2[:, 0:QT],
                        lhsT=kT_sb[ch][0:64, kcol:kcol + 128],
                        rhs=qT_sb[ch][0:64, qcol:qcol + QT],
                        start=True, stop=True,
                    )
                    nc.tensor.matmul(
                        s2[:, QT:2 * QT],
                        lhsT=kT_sb[ch][64:128, kcol:kcol + 128],
                        rhs=qT_sb[ch][64:128, qcol:qcol + QT],
                        start=True, stop=True,
                    )
                    # AV lags TWO chunks behind: AV(kk) needs exp(kk) done
                    # plus its ack; with lag 1 the next score pair queues
                    # behind a not-yet-ready AV and the exp stream stalls.
                    if kk >= 2:
                        av(kk - 2, prev[0])
                    p2 = p2p.tile([128, 2 * QT], BF16, tag="p",
                                  name=f"p{ch}_{qt}_{kk}")
                    nc.scalar.activation(p2[:], s2[:], EXP)
                    prev = [prev[1], p2] if kk > 0 else [None, p2]
                    pump(pump_n)
                av(NKK - 2, prev[0])
                av(NKK - 1, prev[1])

                # normalize: aT = U[0:64] / U[64]
                for i, h in ((0, hA), (1, hB)):
                    p0 = (h % 2) * 64
                    rcp = smpool.tile([1, QT], F32, tag="rcp",
                                      name=f"rcp{ch}_{qt}_{i}")
                    nc.vector.reciprocal(rcp[:], U[i][D:D + 1, :])
                    bc = smpool.tile([64, QT], F32, tag="bc",
                                     name=f"bc{ch}_{qt}_{i}")
                    nc.gpsimd.partition_broadcast(bc[:], rcp[:])
                    nc.vector.tensor_mul(
                        aT_sb[ch][p0:p0 + 64, qcol:qcol + QT],
                        U[i][0:D, :], bc[:])

            # qt-outer: after each qt block, its y rows become computable
            # and are queued behind the projection work.
            for qt in range(NQT):
                for ch in range(4):
                    n = 3 if qt == 0 and ch == 0 else (1 if qt == 0 else 2)
                    attn_pair(ch, qt, n)
                for tci in range(4 * qt, 4 * qt + 4):
                    s0, s1 = emit_y(tci)
                    pending.append([lambda: None, s0, s1, True])
            # drain remaining projection + y work
            while work or pending:
                pump(2)

        if reps == 1:
            body(0)
        else:
            with tc.For_i(0, reps, 1) as iv:
                body(iv)

    nc.compile()
    return nc


def make_in_maps(Q, K, V, Wq, Wk, Wv, Wo):
    """Shard + lay out full inputs for the 8 cores."""
    Q = np.asarray(Q, dtype=np.float32)
    K = np.asarray(K, dtype=np.float32)
    V = np.asarray(V, dtype=np.float32)
    Wq = np.asarray(Wq, dtype=np.float32)
    Wk = np.asarray(Wk, dtype=np.float32)
    Wv = np.asarray(Wv, dtype=np.float32)
    Wo = np.asarray(Wo, dtype=np.float32)

    in_maps = []
    for c in range(NCORES):
        b, g = c // 2, c % 2
        rows = slice(g * F, (g + 1) * F)
        in_maps.append({
            "xq": np.ascontiguousarray(Q[b].T).astype(np.float16),
            "xk": np.ascontiguousarray(K[b].T).astype(np.float16),
            "xv": np.ascontiguousarray(V[b].T).astype(np.float16),
            "wq": np.ascontiguousarray(Wq[rows, :].T).astype(np.float16),
            "wk": np.ascontiguousarray(Wk[rows, :].T).astype(np.float16),
            "wv": np.ascontiguousarray(Wv[rows, :].T).astype(np.float16),
            "wo": np.ascontiguousarray(Wo[:, rows].T).astype(np.float16),
        })
    return in_maps


def combine(results, bo):
    """Sum per-core partials + bias -> full [B, S, E] output."""
    bo = np.asarray(bo, dtype=np.float32)
    y = np.zeros((B, S, E), dtype=np.float32)
    for c in range(NCORES):
        y[c // 2] += results[c]["y"].astype(np.float32)
    y += bo[None, None, :]
    return y


def kernel(Q, K, V, Wq, Wk, Wv, Wo, bo):
    from concourse.bass_utils import run_bass_kernel_spmd

    if "nc" not in _CACHE:
        _CACHE["nc"] = build_nc(reps=1)
    nc = _CACHE["nc"]
    in_maps = make_in_maps(Q, K, V, Wq, Wk, Wv, Wo)
    res = run_bass_kernel_spmd(nc, in_maps, core_ids=list(range(NCORES)))
    return combine(res.results, bo)
